# revision 12
# baseline (speedup 1.0000x reference)
"""BitLinear (RMSNorm + int8 act quant + ternary weight quant + GEMM) on 8 TRN2 cores.

Sharding: 2 token-groups x 4 dout-groups. Each core:
  - x shard [4096, 2048] (token-parallel)
  - wT shard [2048, 2048] = weight[og*2048:(og+1)*2048, :].T  (host pre-transposed layout)
  - wsc shard [1024, 2048] = weight[c*1024:(c+1)*1024, :]     (for global mean|w| AllReduce)
  - norm_weight replicated across 128 partitions

Device pipeline per core (v2 — prologue-optimized):
  A: wsc streamed first, Abs+accum on ACT -> partial -> single early AllReduce
     (no warmup AR: the ~50-70us ncfw arming overlaps the wsc DMA + x stats).
  B: x tiles 0..8 stats/quant emitted before weight quant so DVE/ACT work
     through the arming window and qT tiles are banked for the matmul start.
  C: weight quant in 2 ALU ops per [128,512] chunk:
       c  = clip(w, -ws, +ws)                (DVE/Pool split by k parity)
       wq = bf16(c*inv_ws + 192)             (DVE, RNE to {191,192,193})
     The +192 offset keeps every value exactly representable in bf16 with
     ulp=1 in [128,256).  psum then equals true_mm + 192*rowsum(x_q), fixed
     for free in the PSUM->SBUF copy via activation bias:
       out = psum*alpha + (-192*alpha*rowsum(x_q))
  D: x tiles 9..31 stats interleaved with per-tile matmul blocks; per-token
     scalar chains live on GpSimd (was DVE); output DMA per 512-col chunk.
The quantized GEMM stays exact: x_q in [-127,127], offset weights in
{191,192,193} are bf16-exact, PSUM accumulates fp32 (|partials| ~ 1e6, well
under 2^24 for the signal; the 192-offset adds ~1e-3 relative rounding worst
case, far inside the 2e-2 gate).
"""

import sys

if "/opt/trn_rl_repo" not in sys.path:
    sys.path.insert(0, "/opt/trn_rl_repo")

import numpy as np

# ---------------------------------------------------------------- config

N_CORES = 8
TG, OG = 2, 4            # token groups x dout groups
B, S, DIN, DOUT = 4, 2048, 2048, 8192
TOKENS = B * S           # 8192
T_SH = TOKENS // TG      # 4096 tokens per core
O_SH = DOUT // OG        # 2048 dout per core
WSC_ROWS = DOUT // N_CORES  # 1024 rows of w per core for the scale pass

P = 128                  # partitions
EPS_NORM = 1e-6
EPS_SCALE = 1e-8
QB = 127.0
C_MAGIC = 12582912.0     # 1.5 * 2^23 : float32 RNE integer-rounding constant
W_OFF = 192.0            # bf16 magic: ints exact (ulp=1) in [128, 256)
N_W = float(DOUT * DIN)  # elements of weight for the global mean

N_EARLY = 3              # x tiles with stats emitted before weight quant
                         # (must not exceed the qT pool depth: deeper early
                         # emission deadlocks the DVE queue on qT slots that
                         # only free once matmuls consume them)


def build_bass(t_sh=T_SH, din=DIN, o_sh=O_SH, wsc_rows=WSC_ROWS, n_w=N_W,
               n_cores=N_CORES, group=8):
    """Build the per-core SPMD Bass graph. Shapes parametrized for sim tests."""
    import concourse.bass as bass
    import concourse.bacc as bacc
    import concourse.mybir as mybir
    from concourse import tile

    fp32 = mybir.dt.float32
    bf16 = mybir.dt.bfloat16
    Alu = mybir.AluOpType
    Act = mybir.ActivationFunctionType

    t_tiles = t_sh // P          # token tiles
    k_tiles = din // P           # contraction tiles
    oc_sz = 512 if o_sh >= 512 else o_sh
    oc_chunks = o_sh // oc_sz    # PSUM output chunks per token tile
    wsc_tiles = wsc_rows // P
    n_early = min(N_EARLY, t_tiles)

    nc = bacc.Bacc("TRN2", target_bir_lowering=False, debug=False,
                   num_devices=n_cores)

    x_d = nc.dram_tensor("x", [t_sh, din], fp32, kind="ExternalInput")
    wt_d = nc.dram_tensor("wt", [din, o_sh], fp32, kind="ExternalInput")
    wsc_d = nc.dram_tensor("wsc", [wsc_rows, din], fp32, kind="ExternalInput")
    gw_d = nc.dram_tensor("gw", [P, din], fp32, kind="ExternalInput")
    out_d = nc.dram_tensor("out", [t_sh, o_sh], fp32, kind="ExternalOutput")

    # collective bounce buffers (internal DRAM)
    pin_d = nc.dram_tensor("cc_in", [P, 1], fp32)
    pout_d = nc.dram_tensor("cc_out", [P, 1], fp32)

    with tile.TileContext(nc) as tc:
        with (
            tc.tile_pool(name="persist", bufs=1) as persist,
            tc.tile_pool(name="wscin", bufs=2) as wsc_pool,
            tc.tile_pool(name="xin", bufs=2) as xin_pool,
            tc.tile_pool(name="ybuf", bufs=2) as y_pool,
            tc.tile_pool(name="t1buf", bufs=2) as t1_pool,
            tc.tile_pool(name="qbuf", bufs=2) as q_pool,
            tc.tile_pool(name="qtbuf", bufs=4) as qt_pool,
            tc.tile_pool(name="obuf", bufs=4) as out_pool,
            tc.tile_pool(name="wtq", bufs=6) as wtq_pool,
            tc.tile_pool(name="small", bufs=4) as small,
            tc.tile_pool(name="psum", bufs=8, space="PSUM") as psum_pool,
        ):
            # ---------------- persistent tiles
            gw_sb = persist.tile([P, din], fp32)
            ones_sb = persist.tile([P, P], fp32)
            # per-k quantized transposed weight blocks [d_lo, o], offset +192
            wq = [persist.tile([P, o_sh], bf16, name=f"wq{k}") for k in range(k_tiles)]
            # per-token stats, one column per token tile
            sumsq_t = persist.tile([P, t_tiles], fp32)
            amax_t = persist.tile([P, t_tiles], fp32)
            m_t = persist.tile([P, t_tiles], fp32)
            alpha_t = persist.tile([P, t_tiles], fp32)
            bias_t = persist.tile([P, t_tiles], fp32)   # -192*alpha*rowsum(q)
            wacc = persist.tile([P, wsc_tiles], fp32)

            # ---------------- pass A: global sum |w| -> single early AllReduce
            for j in range(wsc_tiles):
                wtile = wsc_pool.tile([P, din], fp32, tag="wsc")
                nc.scalar.dma_start(wtile[:], wsc_d[j * P:(j + 1) * P, :])
                scr = t1_pool.tile([P, din], fp32, tag="t1")
                nc.scalar.activation(scr[:], wtile[:], Act.Abs,
                                     accum_out=wacc[:, j:j + 1])
            wpart = small.tile([P, 1], fp32)
            nc.vector.tensor_reduce(out=wpart[:], in_=wacc[:], op=Alu.add,
                                    axis=mybir.AxisListType.X)
            nc.scalar.dma_start(pin_d[:], wpart[:])
            nc.gpsimd.collective_compute(
                "AllReduce", Alu.add,
                replica_groups=[list(range(n_cores))],
                ins=[pin_d[:]], outs=[pout_d[:]],
            )
            wsum_all = small.tile([P, 1], fp32)
            nc.scalar.dma_start(wsum_all[:], pout_d[:])
            nc.gpsimd.memset(ones_sb[:], 1.0)
            nc.scalar.dma_start(gw_sb[:], gw_d[:])
            # cross-partition sum + broadcast via ones matmul
            psum_s = psum_pool.tile([P, oc_sz], fp32, tag="ps", name="psum_s")
            nc.tensor.matmul(psum_s[:, 0:1], ones_sb[:], wsum_all[:],
                             start=True, stop=True)
            ssum = small.tile([P, 1], fp32)
            nc.vector.tensor_copy(ssum[:], psum_s[:, 0:1])
            ws = small.tile([P, 1], fp32)   # w_scale per partition (all equal)
            nc.gpsimd.tensor_scalar(out=ws[:], in0=ssum[:], scalar1=1.0 / n_w,
                                    scalar2=EPS_SCALE, op0=Alu.mult, op1=Alu.add)
            neg_ws = small.tile([P, 1], fp32)
            nc.gpsimd.tensor_scalar(out=neg_ws[:], in0=ws[:], scalar1=-1.0,
                                    scalar2=None, op0=Alu.mult)
            inv_ws = small.tile([P, 1], fp32)
            nc.vector.reciprocal(inv_ws[:], ws[:])

            # ---------------- per-tile stat/quant chain (no matmuls)
            def stats(i):
                xt = xin_pool.tile([P, din], fp32, tag="xin")
                nc.scalar.dma_start(xt[:], x_d[i * P:(i + 1) * P, :])
                yt = y_pool.tile([P, din], fp32, tag="y")
                nc.vector.tensor_tensor(out=yt[:], in0=xt[:], in1=gw_sb[:],
                                        op=Alu.mult)
                scr = t1_pool.tile([P, din], fp32, tag="t1")
                nc.scalar.activation(scr[:], xt[:], Act.Square,
                                     accum_out=sumsq_t[:, i:i + 1])
                nc.vector.tensor_reduce(out=amax_t[:, i:i + 1], in_=yt[:],
                                        op=Alu.max, axis=mybir.AxisListType.X,
                                        apply_absolute_value=True)
                # per-token scalars on [P, 1] (GpSimd; reciprocals on DVE)
                mse = small.tile([P, 1], fp32, tag="mse")
                nc.gpsimd.tensor_scalar(out=mse[:], in0=sumsq_t[:, i:i + 1],
                                        scalar1=1.0 / din, scalar2=EPS_NORM,
                                        op0=Alu.mult, op1=Alu.add)
                sq = small.tile([P, 1], fp32, tag="sq")
                nc.scalar.activation(sq[:], mse[:], Act.Sqrt)
                d1 = small.tile([P, 1], fp32, tag="d1")
                nc.gpsimd.tensor_scalar(out=d1[:], in0=amax_t[:, i:i + 1],
                                        scalar1=1.0 / QB, scalar2=None,
                                        op0=Alu.mult)
                # f1 = d1 + EPS_SCALE*sq ; m = 1/f1
                e1 = small.tile([P, 1], fp32, tag="e1")
                nc.gpsimd.tensor_scalar(out=e1[:], in0=sq[:], scalar1=EPS_SCALE,
                                        scalar2=None, op0=Alu.mult)
                f1 = small.tile([P, 1], fp32, tag="f1")
                nc.gpsimd.tensor_tensor(out=f1[:], in0=d1[:], in1=e1[:],
                                        op=Alu.add)
                nc.vector.reciprocal(m_t[:, i:i + 1], f1[:])
                rsq = small.tile([P, 1], fp32, tag="rsq")
                nc.vector.reciprocal(rsq[:], sq[:])
                xs0 = small.tile([P, 1], fp32, tag="xs0")
                nc.gpsimd.tensor_tensor(out=xs0[:], in0=d1[:], in1=rsq[:],
                                        op=Alu.mult)
                # alpha = (xs0 + eps) * w_scale
                nc.gpsimd.tensor_scalar(out=alpha_t[:, i:i + 1], in0=xs0[:],
                                        scalar1=EPS_SCALE, scalar2=ws[:],
                                        op0=Alu.add, op1=Alu.mult)
                # quantize x (fp32 magic round): q = round(yt * m)
                t1 = t1_pool.tile([P, din], fp32, tag="t1")
                nc.vector.tensor_scalar(out=t1[:], in0=yt[:],
                                        scalar1=m_t[:, i:i + 1],
                                        scalar2=C_MAGIC,
                                        op0=Alu.mult, op1=Alu.add)
                # qt8 pass also accumulates rowsum(q) (for the +192
                # weight-offset correction) via accum_out — zero extra cost
                qt8 = q_pool.tile([P, din], bf16, tag="q")
                rs = small.tile([P, 1], fp32, tag="rs")
                nc.vector.tensor_scalar(out=qt8[:], in0=t1[:], scalar1=C_MAGIC,
                                        scalar2=0.0, op0=Alu.subtract,
                                        op1=Alu.add, accum_out=rs[:])
                nc.gpsimd.tensor_scalar(out=bias_t[:, i:i + 1], in0=rs[:],
                                        scalar1=alpha_t[:, i:i + 1],
                                        scalar2=-W_OFF,
                                        op0=Alu.mult, op1=Alu.mult)
                # one xbar transpose for the whole tile: out[d_lo, k, t] =
                # qt8[t, 128k + d_lo]  (verified blocked layout on HW)
                qT = qt_pool.tile([P, k_tiles, P], bf16, tag="qT")
                nc.sync.dma_start(out=qT[:], in_=qt8[:], transpose=True)
                return qT

            qts = {}
            for i in range(n_early):
                qts[i] = stats(i)

            # ---------------- pass C: quantize wT -> {191,192,193} bf16
            # 2 ALU ops per [128, oc_sz] chunk; clip split DVE/Pool by k parity
            for oc in range(oc_chunks):
                osl = slice(oc * oc_sz, (oc + 1) * oc_sz)
                for k in range(k_tiles):
                    wtile = wtq_pool.tile([P, oc_sz], fp32, tag="wtq")
                    nc.scalar.dma_start(wtile[:], wt_d[k * P:(k + 1) * P, osl])
                    ctile = wtq_pool.tile([P, oc_sz], fp32, tag="wclip", bufs=4)
                    eng = nc.vector if (k % 2 == 0) else nc.gpsimd
                    eng.tensor_scalar(out=ctile[:], in0=wtile[:],
                                      scalar1=ws[:], scalar2=neg_ws[:],
                                      op0=Alu.min, op1=Alu.max)
                    nc.vector.tensor_scalar(out=wq[k][:, osl], in0=ctile[:],
                                            scalar1=inv_ws[:], scalar2=W_OFF,
                                            op0=Alu.mult, op1=Alu.add)

            # ---------------- matmul + output block for one token tile
            def mm(i, qT):
                for oc in range(oc_chunks):
                    osl = slice(oc * oc_sz, (oc + 1) * oc_sz)
                    pt = psum_pool.tile([P, oc_sz], fp32, tag="ps")
                    for k in range(k_tiles):
                        nc.tensor.matmul(pt[:], qT[:, k, :], wq[k][:, osl],
                                         start=(k == 0), stop=(k == k_tiles - 1))
                    osb = out_pool.tile([P, oc_sz], fp32, tag="o")
                    nc.scalar.activation(osb[:], pt[:], Act.Identity,
                                         scale=alpha_t[:, i:i + 1],
                                         bias=bias_t[:, i:i + 1])
                    nc.scalar.dma_start(out_d[i * P:(i + 1) * P, osl], osb[:])

            # ---------------- steady state: interleave remaining stats with mms
            for i in range(t_tiles):
                j = i + n_early
                if j < t_tiles:
                    qts[j] = stats(j)
                mm(i, qts.pop(i))

    nc.compile()
    return nc


# ---------------------------------------------------------------- host wrapper

_CACHED = {}


def _get_nc():
    if "nc" not in _CACHED:
        _CACHED["nc"] = build_bass()
    return _CACHED["nc"]


def kernel(x: np.ndarray, weight: np.ndarray, norm_weight: np.ndarray) -> np.ndarray:
    from concourse.bass_utils import run_bass_kernel_spmd

    assert x.shape == (B, S, DIN) and weight.shape == (DOUT, DIN)
    x_flat = np.ascontiguousarray(x.reshape(TOKENS, DIN), dtype=np.float32)
    w = np.ascontiguousarray(weight, dtype=np.float32)
    wt_full = np.ascontiguousarray(w.T)  # [DIN, DOUT]
    gw = np.ascontiguousarray(
        np.broadcast_to(norm_weight.astype(np.float32), (P, DIN)))

    in_maps = []
    for c in range(N_CORES):
        tg, og = divmod(c, OG)
        in_maps.append({
            "x": np.ascontiguousarray(x_flat[tg * T_SH:(tg + 1) * T_SH]),
            "wt": np.ascontiguousarray(wt_full[:, og * O_SH:(og + 1) * O_SH]),
            "wsc": np.ascontiguousarray(w[c * WSC_ROWS:(c + 1) * WSC_ROWS]),
            "gw": gw,
        })

    nc = _get_nc()
    res = run_bass_kernel_spmd(nc, in_maps, core_ids=list(range(N_CORES)))
    _CACHED["last_results"] = res

    out = np.empty((TOKENS, DOUT), dtype=np.float32)
    for c in range(N_CORES):
        tg, og = divmod(c, OG)
        out[tg * T_SH:(tg + 1) * T_SH, og * O_SH:(og + 1) * O_SH] = \
            res.results[c]["out"]
    return out.reshape(B, S, DOUT)
